# revision 1
# baseline (speedup 1.0000x reference)
"""YOLO-style detection decode (nms_detection) on 8 trn2 NeuronCores.

Data-parallel over batch (64 -> 8 images/core). All per-core inputs are
packed into ONE flat f32 DRAM tensor (x52|x26|x13 in natural [b,ch,s]
order, then small constants) and the result is ONE [28392, 18] f32
tensor (cells x (3 anchors x 6)), reassembled on the host. One input +
one output minimizes the large per-tensor dispatch overhead of the
execution path.

Device pipeline per 4-chunk group (chunk = 128 cells):
  - segment DMAs load [128ch, cells] strips (raw channel order).
  - PE transposes chunks into PSUM -> [cell, 255ch].
  - DVE reduce_max over the 80 class cols per anchor -> m (exact).
  - PE transposes m; an fp32 K=3 matmul subtracts m from the class
    logits (exact: Sterbenz near the max) and a K=1 matmul adds
    (79-c)*2^-31. The winner's value is then exactly
    (79-argmax)*2^-31 >= 0 while every loser stays < 0, so a second
    DVE reduce_max recovers argmax exactly (incl. first-index ties,
    matching jnp.argmax).
  - decode: conf = sigmoid (ACT), cx/cy fused scalar_tensor_tensor with
    host grid offsets, w/h = exp * anchors/416, mask = (logit > 0)
    applied multiplicatively (fused is_gt*mult per anchor).
"""

import os
from contextlib import ExitStack

import numpy as np

import concourse.bass as bass
import concourse.tile as tile
from concourse import bacc, mybir
from concourse.bass_utils import run_bass_kernel_spmd

N_CORES = 8
B = 64
B_PER = B // N_CORES
CASE = 416.0
SCALES = [("52", 52, 8.0), ("26", 26, 16.0), ("13", 13, 32.0)]
CHUNK = 128
GRP = 4
F32 = mybir.dt.float32
AX = mybir.AxisListType
OP = mybir.AluOpType
AF = mybir.ActivationFunctionType
IOTA_SCALE = 2.0 ** -31


def _cells(h):
    return B_PER * h * h


def _nchunks(h):
    return (_cells(h) + CHUNK - 1) // CHUNK


def _gxy_section(h, t):
    n = _cells(h)
    nch = _nchunks(h)
    cells = np.arange(nch * CHUNK)
    s = cells % (h * h)
    gx = (s % h).astype(np.float64) * t / CASE
    gy = (s // h).astype(np.float64) * t / CASE
    gx[cells >= n] = 0.0
    gy[cells >= n] = 0.0
    out = np.zeros((CHUNK, 2 * nch), np.float32)
    for j in range(nch):
        out[:, 2 * j] = gx[j * CHUNK:(j + 1) * CHUNK]
        out[:, 2 * j + 1] = gy[j * CHUNK:(j + 1) * CHUNK]
    return out


def _consts():
    import ml_dtypes
    bf = ml_dtypes.bfloat16
    # raw channel order: anchor a's class cols at 85a+5 .. 85a+85.
    # sel9 rows 32q + (3*term + a): -1 selector for the 3-term bf16 split.
    sel9 = np.zeros((128, 256), bf)
    for q in range(4):
        for r in range(9):
            a = r % 3
            sel9[32 * q + r, 85 * a + 5:85 * a + 85] = -1.0
    iotam = np.zeros((1, 256), bf)
    for a in range(3):
        iotam[0, 85 * a + 5:85 * a + 85] = \
            ((79.0 - np.arange(80)) * IOTA_SCALE).astype(bf)
    onesb = np.ones((1, 128), bf)
    iden = np.eye(128, dtype=np.float32)
    gxy = np.concatenate([_gxy_section(h, t) for _, h, t in SCALES], axis=1)
    return {
        "gxy": gxy.astype(np.float32),
        "iden": iden,
        "sel9": sel9.view(np.float32),
        "iotam": iotam.view(np.float32),
        "onesb": onesb.view(np.float32),
    }


_CONSTS = _consts()

# packed input layout (f32 elements, per core)
_X_OFF = {}
_off = 0
for _tag, _h, _t in SCALES:
    _X_OFF[_tag] = _off
    _off += B_PER * 255 * _h * _h
_CONST_OFF = {}
for _name in ("gxy", "iden", "sel9", "iotam", "onesb"):
    _CONST_OFF[_name] = _off
    _off += _CONSTS[_name].size
_CONST_OFF["anch"] = _off
_off += 128 * 18
TOTAL_IN = _off

_O_OFF = {}
_off = 0
for _tag, _h, _t in SCALES:
    _O_OFF[_tag] = _off
    _off += _cells(_h)
TOTAL_OUT_ROWS = _off  # 28392


def _a85(ap_pgx, lo, width=1):
    """[128, gc, 3(anchor), width] view of box channel `lo` from a
    [128, gc, 512] psum group view (channel stride 85)."""
    v = ap_pgx[:, :, 0:255].rearrange("p g (a r) -> p g a r", a=3, r=85)
    return v[:, :, :, lo:lo + width]


def _emit_scale(nc, tc, ctx, pools, sb, xin, oX, h, t, tag, gxy_off):
    ST = int(os.environ.get("KSTAGE", "9"))
    n = _cells(h)
    hw = h * h
    nch = _nchunks(h)
    ngrp = (nch + GRP - 1) // GRP
    k = float(t / CASE)
    (p_ina, p_inb, p_ps, p_m, p_mt, p_out) = pools

    xoff = _X_OFF[tag]
    xr3 = xin[xoff:xoff + B_PER * 255 * hw] \
        .rearrange("(b c s) -> c b s", b=B_PER, c=255)

    def seg_dma(dst_tile, nrows, src0, c0, w):
        done = 0
        while done < w:
            cell = c0 + done
            b = cell // hw
            s = cell % hw
            span = min(w - done, hw - s)
            nc.sync.dma_start(dst_tile[0:nrows, done:done + span],
                              xr3[src0:src0 + nrows, b, s:s + span])
            done += span

    for g in range(ngrp):
        j0 = g * GRP
        gc = min(GRP, nch - j0)
        c0 = j0 * CHUNK
        w = min(GRP * CHUNK, n - c0)

        in_a = p_ina.tile([128, GRP * CHUNK], F32, tag="in_a")
        in_b = p_inb.tile([128, GRP * CHUNK], F32, tag="in_b")
        seg_dma(in_a, 128, 0, c0, w)
        seg_dma(in_b, 127, 128, c0, w)

        ps = p_ps.tile([128, 4 * 512], F32, tag="ps")
        pg = ps[:].rearrange("p (g x) -> p g x", g=4)[:, 0:gc, :]
        ncs = []
        for jj in range(gc):
            ncj = min(CHUNK, w - jj * CHUNK)
            ncs.append(ncj)
            if ncj < CHUNK:
                nc.vector.memset(ps[:, jj * 512:jj * 512 + 255], 0.0)
            nc.tensor.transpose(ps[0:ncj, jj * 512:jj * 512 + 128],
                                in_a[:, jj * CHUNK:jj * CHUNK + ncj],
                                sb["iden"])
            nc.tensor.matmul(ps[0:ncj, jj * 512 + 128:jj * 512 + 255],
                             in_b[0:127, jj * CHUNK:jj * CHUNK + ncj],
                             sb["iden"][0:127, 0:127],
                             is_transpose=True, start=False, stop=True,
                             skip_group_check=True)

        cls_ap = _a85(pg, 5, 80)          # [128, gc, 3, 80]
        conf_ap = _a85(pg, 0).squeeze(3)  # [128, gc, 3]

        # ---- scan 1: exact class max ----
        m_sb = p_m.tile([128, 12], F32, tag="m_sb")
        m_v = m_sb[:].rearrange("p (g a) -> p g a", g=4)[:, 0:gc, :]
        if ST >= 2:
            nc.vector.tensor_reduce(m_v, cls_ap, axis=AX.X, op=OP.max)
        else:
            nc.vector.memset(m_sb[:, :], 0.0)

        # ---- exact 3-term bf16 split of m (gpsimd, off critical engines):
        # m = h1 + h2 + h3 with every term bf16-representable.
        BF16 = mybir.dt.bfloat16
        hb = p_m.tile([128, 12], BF16, tag="hb")
        hb2 = p_m.tile([128, 12], BF16, tag="hb2")
        r1 = p_m.tile([128, 12], F32, tag="r1")
        msp = p_m.tile([128, 128], F32, tag="msp")
        hb_v = hb[:].rearrange("p (g a) -> p g a", g=4)[:, 0:gc, :]
        hb2_v = hb2[:].rearrange("p (g a) -> p g a", g=4)[:, 0:gc, :]
        r1_v = r1[:].rearrange("p (g a) -> p g a", g=4)[:, 0:gc, :]
        mspv = msp[:].rearrange("p (g r) -> p g r", g=4)
        if ST >= 3:
            nc.vector.memset(msp[:, :], 0.0)
            nc.vector.tensor_copy(hb_v, m_v)
            nc.vector.tensor_copy(mspv[:, 0:gc, 0:3], hb_v)
            nc.vector.tensor_tensor(r1_v, m_v, hb_v, op=OP.subtract)
            nc.vector.tensor_copy(hb2_v, r1_v)
            nc.vector.tensor_copy(mspv[:, 0:gc, 3:6], hb2_v)
            nc.vector.tensor_tensor(mspv[:, 0:gc, 6:9], r1_v, hb2_v,
                                    op=OP.subtract)

        # ---- transpose m-split into psum spare (halves: bases 0/32) ----
        mts = []
        for hh in range((gc + 1) // 2 if ST >= 4 else 0):
            nc.tensor.matmul(ps[0:64, hh * 512 + 256:hh * 512 + 384],
                             msp[:, 64 * hh:64 * hh + 64],
                             sb["iden"][0:128, 0:128],
                             is_transpose=True, start=False, stop=True,
                             skip_group_check=True)
            mt_t = p_mt.tile([64, 128], BF16, tag=f"mtsb{hh}")
            nc.scalar.copy(mt_t[:, :],
                           ps[0:64, hh * 512 + 256:hh * 512 + 384])
            mts.append(mt_t)

        # ---- recenter: cls += -m, then += iota (separate accumulates) --
        for jj in range(gc if ST >= 5 else 0):
            out_cls = ps[:, jj * 512:jj * 512 + 255]
            bp = 32 * (jj % 2)
            nc.tensor.matmul(out_cls, mts[jj // 2][bp:bp + 9, :],
                             sb["sel9"][bp:bp + 9, 0:255],
                             start=False, stop=True, skip_group_check=True)
            nc.tensor.matmul(out_cls, sb["onesb"], sb["iotam"][:, 0:255],
                             start=False, stop=True, skip_group_check=True)

        # ---- scan 2: argmax ----
        idx_sb = p_m.tile([128, 12], F32, tag="idx_sb")
        idx_v = idx_sb[:].rearrange("p (g a) -> p g a", g=4)[:, 0:gc, :]
        if ST >= 6:
            nc.vector.tensor_reduce(idx_v, cls_ap, axis=AX.X, op=OP.max)
        else:
            nc.vector.memset(idx_sb[:, :], 0.0)

        # ---- decode ----
        out4 = p_out.tile([128, GRP * 18], F32, tag="out4")
        if ST < 7:
            nc.vector.memset(out4[:, :], 0.0)
        o4 = out4[:].rearrange("p (g a s) -> p g a s", g=4, a=3)
        o4t = out4[:].rearrange("p (g a s) -> p g s a", g=4, a=3)

        if ST >= 7:
            # conf = 1/(1 + exp(-logit)): stay in the Exp table set
            econf = p_m.tile([128, 12], F32, tag="econf")
            e_v = econf[:].rearrange("p (g a) -> p g a", g=4)[:, 0:gc, :]
            nc.scalar.activation(e_v, conf_ap, AF.Exp, scale=-1.0)
            ep1 = p_m.tile([128, 12], F32, tag="ep1")
            e1_v = ep1[:].rearrange("p (g a) -> p g a", g=4)[:, 0:gc, :]
            nc.vector.tensor_scalar(e1_v, e_v, 1.0, None, op0=OP.add)
            nc.vector.reciprocal(o4t[:, 0:gc, 0:1, :].squeeze(2), e1_v)

            gxy_ap = sb["gxy"][:, gxy_off + 2 * j0:gxy_off + 2 * j0 + 2 * gc]
            gxy_r = gxy_ap.rearrange("p (g q) -> p g q", q=2)
            for kk in range(2):
                g_v = gxy_r[:, :, kk:kk + 1].broadcast_to([128, gc, 3])
                src = _a85(pg, 1 + kk).squeeze(3)
                dst = o4t[:, 0:gc, 1 + kk:2 + kk, :].squeeze(2)
                nc.vector.scalar_tensor_tensor(dst, src, k, g_v,
                                               op0=OP.mult, op1=OP.add)

            twh = p_m.tile([128, 24], F32, tag="twh")
            twh_v = twh[:].rearrange("p (g q a) -> p g q a", g=4, q=2)
            for kk in range(2):
                nc.scalar.activation(
                    twh_v[:, 0:gc, kk:kk + 1, :].squeeze(2),
                    _a85(pg, 3 + kk).squeeze(3), AF.Exp)
            anch_v = sb["anch"].rearrange("p (q a) -> p q a", q=2) \
                .unsqueeze(1).broadcast_to([128, gc, 2, 3])
            nc.vector.tensor_tensor(o4t[:, 0:gc, 3:5, :],
                                    twh_v[:, 0:gc], anch_v, op=OP.mult)

            nc.scalar.activation(o4t[:, 0:gc, 5:6, :].squeeze(2), idx_v,
                                 AF.Copy, bias=79.0, scale=-(2.0 ** 31))

            for a in range(3):
                cb = conf_ap[:, :, a:a + 1].broadcast_to([128, gc, 6])
                dst = o4[:, 0:gc, a, :]
                nc.vector.scalar_tensor_tensor(dst, cb, 0.0, dst,
                                               op0=OP.is_gt, op1=OP.mult)

        nfull = sum(1 for x in ncs if x == CHUNK)
        r0 = _O_OFF[tag] + c0
        if nfull:
            dst = oX[r0:r0 + nfull * CHUNK, :] \
                .rearrange("(g p) c -> p g c", p=CHUNK)
            nc.sync.dma_start(dst, o4[:, 0:nfull].rearrange(
                "p g a s -> p g (a s)"))
        if nfull < gc:
            ncj = ncs[nfull]
            rp = r0 + nfull * CHUNK
            nc.sync.dma_start(oX[rp:rp + ncj, :],
                              out4[0:ncj, 18 * nfull:18 * nfull + 18])


def build():
    nc = bacc.Bacc("TRN2", target_bir_lowering=False, debug=False,
                   num_devices=N_CORES)
    xin = nc.dram_tensor("xin", [TOTAL_IN], F32, kind="ExternalInput").ap()
    oX = nc.dram_tensor("out", [TOTAL_OUT_ROWS, 18], F32,
                        kind="ExternalOutput").ap()

    with tile.TileContext(nc) as tc:
        with ExitStack() as ctx:
            p_c = ctx.enter_context(tc.tile_pool(name="consts", bufs=1))
            p_ina = ctx.enter_context(tc.tile_pool(name="inpa", bufs=4))
            p_inb = ctx.enter_context(tc.tile_pool(name="inpb", bufs=4))
            p_ps = ctx.enter_context(
                tc.tile_pool(name="ps", bufs=2, space="PSUM"))
            p_m = ctx.enter_context(tc.tile_pool(name="small", bufs=3))
            p_mt = ctx.enter_context(tc.tile_pool(name="mt", bufs=3))
            p_out = ctx.enter_context(tc.tile_pool(name="out", bufs=4))

            shapes = {"gxy": [128, _CONSTS["gxy"].shape[1]],
                      "iden": [128, 128], "sel9": [128, 128],
                      "iotam": [1, 128], "onesb": [1, 64],
                      "anch": [128, 18]}
            sb = {}
            for name, shp in shapes.items():
                t_ = p_c.tile(shp, F32, tag=name)
                size = shp[0] * shp[1]
                src = xin[_CONST_OFF[name]:_CONST_OFF[name] + size] \
                    .rearrange("(p f) -> p f", p=shp[0])
                nc.sync.dma_start(t_[:], src)
                if name in ("sel9", "iotam", "onesb"):
                    sb[name] = t_[:].bitcast(mybir.dt.bfloat16)
                else:
                    sb[name] = t_[:]
            anch_t = sb["anch"]

            pools = (p_ina, p_inb, p_ps, p_m, p_mt, p_out)
            for _rep in range(int(os.environ.get("KREP", "1"))):
                gxy_off = 0
                anch_off = 0
                for tag, h, t in SCALES:
                    sbs = dict(sb)
                    sbs["anch"] = anch_t[:, anch_off:anch_off + 6]
                    _emit_scale(nc, tc, ctx, pools, sbs, xin, oX, h, t,
                                tag, gxy_off)
                    gxy_off += 2 * _nchunks(h)
                    anch_off += 6
    nc.compile()
    return nc


_NC = None


def _get_nc():
    global _NC
    if _NC is None:
        _NC = build()
    return _NC


def _make_anch(anchors):
    anch = np.zeros((128, 18), np.float32)
    off = 0
    for tag, h, _ in SCALES:
        a = anchors[tag].astype(np.float64) / CASE
        for kk in range(2):
            for aa in range(3):
                anch[:, off + kk * 3 + aa] = a[aa, kk]
        off += 6
    return anch


def _pack_core(xs, anch):
    parts = [np.asarray(xs["52"]).ravel(), np.asarray(xs["26"]).ravel(),
             np.asarray(xs["13"]).ravel(),
             _CONSTS["gxy"].ravel(), _CONSTS["iden"].ravel(),
             _CONSTS["sel9"].ravel(), _CONSTS["iotam"].ravel(),
             _CONSTS["onesb"].ravel(), anch.ravel()]
    out = np.concatenate(parts)
    assert out.size == TOTAL_IN and out.dtype == np.float32
    return out


def kernel(out13, out26, out52, anchors13, anchors26, anchors52):
    nc = _get_nc()
    xs_all = {"13": np.asarray(out13), "26": np.asarray(out26),
              "52": np.asarray(out52)}
    anchors = {"13": np.asarray(anchors13), "26": np.asarray(anchors26),
               "52": np.asarray(anchors52)}
    anch = _make_anch(anchors)

    in_maps = []
    for i in range(N_CORES):
        xs = {tag: xs_all[tag][i * B_PER:(i + 1) * B_PER]
              for tag, _, _ in SCALES}
        in_maps.append({"xin": _pack_core(xs, anch)})

    res = run_bass_kernel_spmd(nc, in_maps, list(range(N_CORES))).results

    parts = []
    for tag, h, _ in SCALES[::-1]:  # output order: 13, 26, 52
        o0 = _O_OFF[tag]
        for i in range(N_CORES):
            parts.append(res[i]["out"][o0:o0 + _cells(h)].reshape(-1, 6))
    return np.concatenate(parts, axis=0)



# revision 4
# speedup vs baseline: 346.9746x; 346.9746x over previous
"""YOLO-style detection decode (nms_detection) on 8 trn2 NeuronCores.

Data-parallel over batch (64 -> 8 images/core). The host repacks each
core's inputs into cell-major rows so the device needs NO transposes:

  row[cell] (256 f32, 64B-aligned) =
    [conf a0..a2 | x a0..a2 | y a0..a2 | w a0..a2 | h a0..a2 |
     cls a0 c0..c79 | cls a1 | cls a2 | pad]

Cells are padded per scale to whole 128-cell chunks; chunks are grouped
G at a time and stored [group][partition][G*256] so each group loads
with ONE DMA of 128 contiguous ~16KB packets (the baseline was DMA
descriptor-bound: 51k packets averaging 573B).

Device pipeline per group (no PE, no PSUM):
  - DVE  : m  = segmented reduce_max over each anchor's 80 class cols
  - Pool : eq = (cls >= m) -> bf16 {0,1}   (exact: m is a member value)
  - DVE  : eq *= iota (bf16 79-c)  ;  m2 = reduce_max(eq)  -> 79-argmax
           (first-index ties win automatically: larger 79-c)
  - ACT  : conf = Sigmoid(row0..2), ewh = Exp(w,h), cls = 79 - m2
  - DVE  : cx/cy fused stt with host-precomputed gx*t/416 grid,
           w/h = ewh * anchors/416, mask = (conf_logit > 0) * row
  - results accumulate in one SBUF-resident [128, 223*18] tile,
    flushed with a single 128x16KB DMA at the end.
"""

import math
import os
from contextlib import ExitStack

import numpy as np

import concourse.bass as bass
import concourse.tile as tile
from concourse import bacc, mybir
from concourse.bass_utils import run_bass_kernel_spmd

N_CORES = 8
B = 64
B_PER = B // N_CORES
CASE = 416.0
SCALES = [("52", 52, 8.0), ("26", 26, 16.0), ("13", 13, 32.0)]
CHUNK = 128
ROW = 256          # padded row width (f32) per cell
G = int(os.environ.get("KGRP", "16"))
F32 = mybir.dt.float32
BF16 = mybir.dt.bfloat16
AX = mybir.AxisListType
OP = mybir.AluOpType
AF = mybir.ActivationFunctionType

# channel permutation raw->row: [conf*3, x*3, y*3, w*3, h*3, cls a-major]
_PERM = ([85 * a + f for f in range(5) for a in range(3)]
         + [85 * a + 5 + c for a in range(3) for c in range(80)])


def _cells(h):
    return B_PER * h * h


def _nch(h):
    return (_cells(h) + CHUNK - 1) // CHUNK


NCH = {tag: _nch(h) for tag, h, _ in SCALES}
TOT_CH = sum(NCH.values())                      # 223
CH_OFF = {}
_c = 0
for _tag, _h, _t in SCALES:
    CH_OFF[_tag] = _c
    _c += NCH[_tag]

# groups: (tag, scale_idx, j0, gc, x_off_elems)
GROUPS = []
_off = 0
for _si, (_tag, _h, _t) in enumerate(SCALES):
    for _j0 in range(0, NCH[_tag], G):
        _gc = min(G, NCH[_tag] - _j0)
        GROUPS.append((_tag, _si, _j0, _gc, _off))
        _off += CHUNK * _gc * ROW
_X_SIZE = _off

_GXY_OFF = {}
for _tag, _h, _t in SCALES:
    _GXY_OFF[_tag] = _off
    _off += CHUNK * NCH[_tag] * 2
_ANCH_OFF = _off
_off += CHUNK * 18
_IOTA_OFF = _off
_off += CHUNK * 120                              # 240 bf16 packed as 120 f32
TOTAL_IN = _off
OUT_COLS = TOT_CH * 18


def _gxy_scale(h, t):
    """[128, nch*2] f32: per (partition, chunk): gx*t/416, gy*t/416."""
    hw = h * h
    n = _cells(h)
    nch = NCH[{52: "52", 26: "26", 13: "13"}[h]]
    cells = np.arange(nch * CHUNK)
    s = cells % hw
    gx = (s % h).astype(np.float64) * t / CASE
    gy = (s // h).astype(np.float64) * t / CASE
    gx[cells >= n] = 0.0
    gy[cells >= n] = 0.0
    out = np.zeros((CHUNK, nch, 2), np.float32)
    out[:, :, 0] = gx.reshape(nch, CHUNK).T
    out[:, :, 1] = gy.reshape(nch, CHUNK).T
    return out.reshape(CHUNK, nch * 2)


def _consts():
    import ml_dtypes
    iota = np.zeros((CHUNK, 240), ml_dtypes.bfloat16)
    iota[:, :] = np.tile((79.0 - np.arange(80)), 3)[None, :]
    return {tag: _gxy_scale(h, t) for tag, h, t in SCALES} | {
        "iota": iota.view(np.float32)}


_CONSTS = _consts()


def build():
    nc = bacc.Bacc("TRN2", target_bir_lowering=False, debug=False,
                   num_devices=N_CORES)
    xin = nc.dram_tensor("xin", [TOTAL_IN], F32, kind="ExternalInput").ap()
    oX = nc.dram_tensor("out", [CHUNK, OUT_COLS], F32,
                        kind="ExternalOutput").ap()

    with tile.TileContext(nc) as tc:
        with ExitStack() as ctx:
            p_c = ctx.enter_context(tc.tile_pool(name="consts", bufs=1))
            p_in = ctx.enter_context(tc.tile_pool(name="inp", bufs=3))
            p_eq = ctx.enter_context(tc.tile_pool(name="eq", bufs=3))
            p_s = ctx.enter_context(tc.tile_pool(name="small", bufs=3))

            def load_const(name, cols, off):
                t_ = p_c.tile([CHUNK, cols], F32, tag=name)
                nc.sync.dma_start(
                    t_[:], xin[off:off + CHUNK * cols]
                    .rearrange("(p f) -> p f", p=CHUNK))
                return t_

            gxy_t = {tag: load_const(f"gxy{tag}", NCH[tag] * 2,
                                     _GXY_OFF[tag]) for tag, _, _ in SCALES}
            anch_t = load_const("anch", 18, _ANCH_OFF)
            iota_t = load_const("iota", 120, _IOTA_OFF)
            iota_v = iota_t[:].bitcast(BF16) \
                .rearrange("p (a r) -> p a r", a=3)

            out_t = p_c.tile([CHUNK, OUT_COLS], F32, tag="out_t")
            o_all = out_t[:].rearrange("p (ch a s) -> p ch a s", a=3, s=6)

            for tag, si, j0, gc, xoff in GROUPS:
                _, h, t = SCALES[si]
                k = float(t / CASE)

                strip = p_in.tile([CHUNK, G * ROW], F32, tag="strip")
                nc.sync.dma_start(
                    strip[0:CHUNK, 0:gc * ROW],
                    xin[xoff:xoff + CHUNK * gc * ROW]
                    .rearrange("(p f) -> p f", p=CHUNK))
                sv = strip[:].rearrange("p (g c) -> p g c", g=G)[:, 0:gc]
                cls_ap = sv[:, :, 15:255].rearrange(
                    "p g (a r) -> p g a r", a=3)

                m = p_s.tile([CHUNK, G * 3], F32, tag="m")
                m_v = m[:].rearrange("p (g a) -> p g a", g=G)[:, 0:gc]
                nc.vector.tensor_reduce(m_v, cls_ap, axis=AX.X, op=OP.max)

                eq = p_eq.tile([CHUNK, G * 240], BF16, tag="eq")
                eq_v = eq[:].rearrange("p (g a r) -> p g a r",
                                       g=G, a=3)[:, 0:gc]
                m_b = m_v.unsqueeze(3).broadcast_to([CHUNK, gc, 3, 80])
                nc.vector.tensor_tensor(eq_v, cls_ap, m_b, op=OP.is_ge)

                iota_b = iota_v.unsqueeze(1).broadcast_to([CHUNK, gc, 3, 80])
                nc.vector.tensor_tensor(eq_v, eq_v, iota_b, op=OP.mult)

                m2 = p_s.tile([CHUNK, G * 3], BF16, tag="m2")
                m2_v = m2[:].rearrange("p (g a) -> p g a", g=G)[:, 0:gc]
                nc.vector.tensor_reduce(m2_v, eq_v, axis=AX.X, op=OP.max)

                o_v = o_all[:, CH_OFF[tag] + j0:CH_OFF[tag] + j0 + gc]

                nc.scalar.activation(o_v[:, :, :, 0], sv[:, :, 0:3],
                                     AF.Sigmoid)

                gxy_r = gxy_t[tag][:].rearrange("p (j q) -> p j q", q=2)
                for d in range(2):
                    g_b = gxy_r[:, j0:j0 + gc, d:d + 1] \
                        .broadcast_to([CHUNK, gc, 3])
                    nc.vector.scalar_tensor_tensor(
                        o_v[:, :, :, 1 + d], sv[:, :, 3 + 3 * d:6 + 3 * d],
                        k, g_b, op0=OP.mult, op1=OP.add)

                ewh = p_s.tile([CHUNK, G * 6], F32, tag="ewh")
                ewh_v = ewh[:].rearrange("p (g d a) -> p g d a",
                                         g=G, d=2)[:, 0:gc]
                nc.scalar.activation(
                    ewh_v, sv[:, :, 9:15].rearrange("p g (d a) -> p g d a",
                                                    d=2), AF.Exp)
                anch_b = anch_t[:, 6 * si:6 * si + 6] \
                    .rearrange("p (d a) -> p d a", d=2) \
                    .rearrange("p d a -> p a d").unsqueeze(1) \
                    .broadcast_to([CHUNK, gc, 3, 2])
                nc.vector.tensor_tensor(
                    o_v[:, :, :, 3:5],
                    ewh_v.rearrange("p g d a -> p g a d"), anch_b,
                    op=OP.mult)

                nc.scalar.activation(o_v[:, :, :, 5], m2_v, AF.Copy,
                                     bias=79.0, scale=-1.0)

                m01 = p_s.tile([CHUNK, G * 3], F32, tag="m01")
                m01_v = m01[:].rearrange("p (g a) -> p g a", g=G)[:, 0:gc]
                nc.vector.tensor_scalar(m01_v, sv[:, :, 0:3], 0.0, None,
                                        op0=OP.is_gt)
                nc.vector.tensor_tensor(
                    o_v, o_v,
                    m01_v.unsqueeze(3).broadcast_to([CHUNK, gc, 3, 6]),
                    op=OP.mult)

            nc.sync.dma_start(oX, out_t[:])
    nc.compile()
    return nc


_NC = None


def _get_nc():
    global _NC
    if _NC is None:
        _NC = build()
    return _NC


def _make_anch(anchors):
    """[128, 18] f32: per scale si, cols 6si+3d+a = anchors[a, d]/416."""
    anch = np.zeros((CHUNK, 18), np.float32)
    for si, (tag, _, _) in enumerate(SCALES):
        a = anchors[tag].astype(np.float64) / CASE
        for d in range(2):
            for aa in range(3):
                anch[:, 6 * si + 3 * d + aa] = a[aa, d]
    return anch


def _pack_core(xs, anch):
    """xs: {tag: [B_PER, 255, h, h] f32}; returns flat [TOTAL_IN] f32."""
    parts = []
    for si, (tag, h, t) in enumerate(SCALES):
        hw = h * h
        n = _cells(h)
        nch = NCH[tag]
        x = np.asarray(xs[tag]).reshape(B_PER, 255, hw)
        rows = np.zeros((nch * CHUNK, ROW), np.float32)
        rows[:n, :255] = x.transpose(0, 2, 1).reshape(n, 255)[:, _PERM]
        a = rows.reshape(nch, CHUNK, ROW)
        for j0 in range(0, nch, G):
            gc = min(G, nch - j0)
            parts.append(np.ascontiguousarray(
                a[j0:j0 + gc].transpose(1, 0, 2)).ravel())
    for tag, _, _ in SCALES:
        parts.append(_CONSTS[tag].ravel())
    parts.append(anch.ravel())
    parts.append(_CONSTS["iota"].ravel())
    out = np.concatenate(parts)
    assert out.size == TOTAL_IN and out.dtype == np.float32
    return out


def _unpack(res):
    """res: list of per-core {"out": [128, OUT_COLS]} -> [681408, 6]."""
    parts = []
    for tag, h, _ in SCALES[::-1]:               # output order: 13, 26, 52
        n = _cells(h)
        nch = NCH[tag]
        c0 = CH_OFF[tag] * 18
        for i in range(N_CORES):
            o = res[i]["out"][:, c0:c0 + nch * 18]
            o = o.reshape(CHUNK, nch, 18).transpose(1, 0, 2) \
                .reshape(nch * CHUNK, 18)[:n]
            parts.append(o.reshape(-1, 6))
    return np.concatenate(parts, axis=0)


def kernel(out13, out26, out52, anchors13, anchors26, anchors52):
    nc = _get_nc()
    xs_all = {"13": np.asarray(out13), "26": np.asarray(out26),
              "52": np.asarray(out52)}
    anchors = {"13": np.asarray(anchors13), "26": np.asarray(anchors26),
               "52": np.asarray(anchors52)}
    anch = _make_anch(anchors)

    in_maps = []
    for i in range(N_CORES):
        xs = {tag: xs_all[tag][i * B_PER:(i + 1) * B_PER]
              for tag, _, _ in SCALES}
        in_maps.append({"xin": _pack_core(xs, anch)})

    res = run_bass_kernel_spmd(nc, in_maps, list(range(N_CORES))).results
    return _unpack(res)


# revision 5
# speedup vs baseline: 349.8379x; 1.0083x over previous
"""YOLO-style detection decode (nms_detection) on 8 trn2 NeuronCores.

Data-parallel over batch (64 -> 8 images/core). The host repacks each
core's inputs into (cell, anchor)-major rows of 85 f32 — which is
exactly the raw per-anchor channel block [conf, x, y, w, h, cls x80] —
so the device needs NO transposes and every operand is a dense <=3D
access pattern:

  DRAM row r = cell*3 + a  (85 f32, 340B)

Rows are padded per scale to whole 128-row chunks; chunks are grouped
G at a time and stored [group][partition][G*85] so each group loads
with ONE DMA of 128 contiguous ~16KB packets (the original kernel was
DMA-descriptor-bound: 51k packets averaging 573B).

Device pipeline per group (no PE, no PSUM, one 128-lane row per
(cell,anchor)):
  - DVE : m  = reduce_max over the 80 class cols     [p, g, 80] f32
  - DVE : eq = (cls >= m) -> bf16 {0,1}  (exact: m is a member value)
  - DVE : eq *= iota (bf16 79-c); m2 = reduce_max(eq) = 79 - argmax
          (first-index ties win automatically: larger 79-c)
  - ACT : conf = Sigmoid(col0), ewh = Exp(cols 3:5)   (per-chunk
          grid offsets gx*t/416 and anchors/416 are shipped per
          (partition, chunk) so no per-scale special casing)
  - DVE : cx/cy fused stt, w/h = ewh*anchors, cls = 79 - m2,
          mask = (conf_logit > 0) * row   (one stt, 3D)
  - results accumulate in one SBUF-resident [128, 666*6] tile,
    flushed with a single 128x16KB DMA at the end.
"""

import os
from contextlib import ExitStack

import numpy as np

import concourse.bass as bass
import concourse.tile as tile
from concourse import bacc, mybir
from concourse.bass_utils import run_bass_kernel_spmd

N_CORES = 8
B = 64
B_PER = B // N_CORES
CASE = 416.0
SCALES = [("52", 52, 8.0), ("26", 26, 16.0), ("13", 13, 32.0)]
CHUNK = 128
ROW = 85           # f32 per (cell, anchor) row
G = int(os.environ.get("KGRP", "48"))
F32 = mybir.dt.float32
BF16 = mybir.dt.bfloat16
AX = mybir.AxisListType
OP = mybir.AluOpType
AF = mybir.ActivationFunctionType


def _cells(h):
    return B_PER * h * h


def _rows(h):
    return 3 * _cells(h)


def _nch(h):
    return (_rows(h) + CHUNK - 1) // CHUNK


NCH = {tag: _nch(h) for tag, h, _ in SCALES}     # 507, 127, 32
TOT_CH = sum(NCH.values())                        # 666
CH_OFF = {}
_c = 0
for _tag, _h, _t in SCALES:
    CH_OFF[_tag] = _c
    _c += NCH[_tag]

# groups: (tag, scale_idx, j0, gc, x_off_elems)
GROUPS = []
_off = 0
for _si, (_tag, _h, _t) in enumerate(SCALES):
    for _j0 in range(0, NCH[_tag], G):
        _gc = min(G, NCH[_tag] - _j0)
        GROUPS.append((_tag, _si, _j0, _gc, _off))
        _off += CHUNK * _gc * ROW
_GXY_OFF = _off
_off += CHUNK * TOT_CH * 2
_ANCH_OFF = _off
_off += CHUNK * TOT_CH * 2
_IOTA_OFF = _off
_off += CHUNK * 40                                # 80 bf16 packed as 40 f32
TOTAL_IN = _off
OUT_COLS = TOT_CH * 6


def _consts():
    """gxy / anch per (partition, global chunk): [128, TOT_CH, 2]."""
    import ml_dtypes
    gxy = np.zeros((CHUNK, TOT_CH, 2), np.float32)
    anch = np.zeros((CHUNK, TOT_CH, 2), np.float32)  # filled at pack time
    for si, (tag, h, t) in enumerate(SCALES):
        hw = h * h
        nr = _rows(h)
        nch = NCH[tag]
        r = np.arange(nch * CHUNK)
        cell = r // 3
        simg = cell % hw
        gx = (simg % h).astype(np.float64) * t / CASE
        gy = (simg // h).astype(np.float64) * t / CASE
        gx[r >= nr] = 0.0
        gy[r >= nr] = 0.0
        j0 = CH_OFF[tag]
        gxy[:, j0:j0 + nch, 0] = gx.reshape(nch, CHUNK).T
        gxy[:, j0:j0 + nch, 1] = gy.reshape(nch, CHUNK).T
    iota = np.zeros((CHUNK, 80), ml_dtypes.bfloat16)
    iota[:, :] = (79.0 - np.arange(80))[None, :]
    return gxy, iota.view(np.float32)


_GXY, _IOTA = _consts()


def _anch_pj(anchors):
    """[128, TOT_CH, 2] f32: anchors[a(r), d] / 416 per (p, chunk)."""
    anch = np.zeros((CHUNK, TOT_CH, 2), np.float32)
    for si, (tag, h, t) in enumerate(SCALES):
        nr = _rows(h)
        nch = NCH[tag]
        a416 = np.asarray(anchors[tag], np.float64) / CASE  # [3, 2]
        r = np.arange(nch * CHUNK)
        av = a416[r % 3]                                    # [nch*128, 2]
        av[r >= nr] = 0.0
        j0 = CH_OFF[tag]
        anch[:, j0:j0 + nch, :] = av.reshape(nch, CHUNK, 2) \
            .transpose(1, 0, 2).astype(np.float32)
    return anch


def build():
    nc = bacc.Bacc("TRN2", target_bir_lowering=False, debug=False,
                   num_devices=N_CORES)
    xin = nc.dram_tensor("xin", [TOTAL_IN], F32, kind="ExternalInput").ap()
    oX = nc.dram_tensor("out", [CHUNK, OUT_COLS], F32,
                        kind="ExternalOutput").ap()

    with tile.TileContext(nc) as tc:
        with ExitStack() as ctx:
            p_c = ctx.enter_context(tc.tile_pool(name="consts", bufs=1))
            p_in = ctx.enter_context(tc.tile_pool(name="inp", bufs=3))
            p_eq = ctx.enter_context(tc.tile_pool(name="eq", bufs=3))
            p_s = ctx.enter_context(tc.tile_pool(name="small", bufs=3))

            def load_const(name, cols, off):
                t_ = p_c.tile([CHUNK, cols], F32, tag=name)
                nc.sync.dma_start(
                    t_[:], xin[off:off + CHUNK * cols]
                    .rearrange("(p f) -> p f", p=CHUNK))
                return t_

            gxy_t = load_const("gxy", TOT_CH * 2, _GXY_OFF)
            anch_t = load_const("anch", TOT_CH * 2, _ANCH_OFF)
            iota_t = load_const("iota", 40, _IOTA_OFF)
            gxy_v = gxy_t[:].rearrange("p (j q) -> p j q", q=2)
            anch_v = anch_t[:].rearrange("p (j q) -> p j q", q=2)
            iota80 = iota_t[:].bitcast(BF16)                # [128, 80]

            out_t = p_c.tile([CHUNK, OUT_COLS], F32, tag="out_t")
            o_all = out_t[:].rearrange("p (ch s) -> p ch s", s=6)

            for tag, si, j0, gc, xoff in GROUPS:
                _, h, t = SCALES[si]
                k = float(t / CASE)
                jg = CH_OFF[tag] + j0                       # global chunk idx

                strip = p_in.tile([CHUNK, G * ROW], F32, tag="strip")
                nc.sync.dma_start(
                    strip[0:CHUNK, 0:gc * ROW],
                    xin[xoff:xoff + CHUNK * gc * ROW]
                    .rearrange("(p f) -> p f", p=CHUNK))
                sv = strip[:].rearrange("p (g c) -> p g c", g=G)[:, 0:gc]
                cls_ap = sv[:, :, 5:85]                     # [p, gc, 80]

                m = p_s.tile([CHUNK, G], F32, tag="m")
                m_v = m[:, 0:gc]
                nc.vector.tensor_reduce(m_v, cls_ap, axis=AX.X, op=OP.max)

                eq = p_eq.tile([CHUNK, G * 80], BF16, tag="eq")
                eq_v = eq[:].rearrange("p (g r) -> p g r", g=G)[:, 0:gc]
                m_b = m_v.unsqueeze(2).broadcast_to([CHUNK, gc, 80])
                nc.vector.tensor_tensor(eq_v, cls_ap, m_b, op=OP.is_ge)

                iota_b = iota80.unsqueeze(1).broadcast_to([CHUNK, gc, 80])
                nc.vector.tensor_tensor(eq_v, eq_v, iota_b, op=OP.mult)

                m2 = p_s.tile([CHUNK, G], BF16, tag="m2")
                m2_v = m2[:, 0:gc]
                nc.vector.tensor_reduce(m2_v, eq_v, axis=AX.X, op=OP.max)

                o_v = o_all[:, jg:jg + gc]                  # [p, gc, 6]

                nc.scalar.activation(o_v[:, :, 0:1].squeeze(2),
                                     sv[:, :, 0:1].squeeze(2), AF.Sigmoid)

                for d in range(2):
                    nc.vector.scalar_tensor_tensor(
                        o_v[:, :, 1 + d:2 + d].squeeze(2),
                        sv[:, :, 1 + d:2 + d].squeeze(2), k,
                        gxy_v[:, jg:jg + gc, d:d + 1].squeeze(2),
                        op0=OP.mult, op1=OP.add)

                ewh = p_s.tile([CHUNK, G * 2], F32, tag="ewh")
                ewh_v = ewh[:].rearrange("p (g q) -> p g q", q=2)[:, 0:gc]
                nc.scalar.activation(ewh_v, sv[:, :, 3:5], AF.Exp)
                nc.vector.tensor_tensor(o_v[:, :, 3:5], ewh_v,
                                        anch_v[:, jg:jg + gc, :], op=OP.mult)

                nc.vector.tensor_scalar(o_v[:, :, 5:6].squeeze(2), m2_v,
                                        -1.0, 79.0, op0=OP.mult, op1=OP.add)

                conf_b = sv[:, :, 0:1].broadcast_to([CHUNK, gc, 6])
                nc.vector.scalar_tensor_tensor(
                    o_v, conf_b, 0.0, o_v, op0=OP.is_gt, op1=OP.mult)

            nc.sync.dma_start(oX, out_t[:])
    nc.compile()
    return nc


_NC = None


def _get_nc():
    global _NC
    if _NC is None:
        _NC = build()
    return _NC


def _make_anch(anchors):
    return _anch_pj(anchors).reshape(CHUNK, TOT_CH * 2)


def _pack_core(xs, anch):
    """xs: {tag: [B_PER, 255, h, h] f32}; anch: [128, TOT_CH*2] f32."""
    parts = []
    for si, (tag, h, t) in enumerate(SCALES):
        hw = h * h
        nr = _rows(h)
        nch = NCH[tag]
        x = np.asarray(xs[tag]).reshape(B_PER, 255, hw)
        rows = np.zeros((nch * CHUNK, ROW), np.float32)
        rows[:nr] = x.transpose(0, 2, 1).reshape(nr, ROW)
        a = rows.reshape(nch, CHUNK, ROW)
        for j0 in range(0, nch, G):
            gc = min(G, nch - j0)
            parts.append(np.ascontiguousarray(
                a[j0:j0 + gc].transpose(1, 0, 2)).ravel())
    parts.append(_GXY.ravel())
    parts.append(np.asarray(anch, np.float32).ravel())
    parts.append(_IOTA.ravel())
    out = np.concatenate(parts)
    assert out.size == TOTAL_IN and out.dtype == np.float32
    return out


def _unpack(res):
    """res: list of per-core {"out": [128, OUT_COLS]} -> [681408, 6]."""
    parts = []
    for tag, h, _ in SCALES[::-1]:               # output order: 13, 26, 52
        nr = _rows(h)
        nch = NCH[tag]
        c0 = CH_OFF[tag] * 6
        for i in range(N_CORES):
            o = res[i]["out"][:, c0:c0 + nch * 6]
            parts.append(o.reshape(CHUNK, nch, 6).transpose(1, 0, 2)
                         .reshape(nch * CHUNK, 6)[:nr])
    return np.concatenate(parts, axis=0)


def kernel(out13, out26, out52, anchors13, anchors26, anchors52):
    nc = _get_nc()
    xs_all = {"13": np.asarray(out13), "26": np.asarray(out26),
              "52": np.asarray(out52)}
    anchors = {"13": np.asarray(anchors13), "26": np.asarray(anchors26),
               "52": np.asarray(anchors52)}
    anch = _make_anch(anchors)

    in_maps = []
    for i in range(N_CORES):
        xs = {tag: xs_all[tag][i * B_PER:(i + 1) * B_PER]
              for tag, _, _ in SCALES}
        in_maps.append({"xin": _pack_core(xs, anch)})

    res = run_bass_kernel_spmd(nc, in_maps, list(range(N_CORES))).results
    return _unpack(res)


# revision 8
# speedup vs baseline: 392.3851x; 1.1216x over previous
"""YOLO-style detection decode (nms_detection) on 8 trn2 NeuronCores.

Data-parallel over batch (64 -> 8 images/core). The host repacks each
core's inputs into (cell, anchor)-major rows of 85 f32 — which is
exactly the raw per-anchor channel block [conf, x, y, w, h, cls x80] —
so the device needs NO transposes and every operand is a dense <=3D
access pattern:

  DRAM row r = cell*3 + a  (85 f32, 340B)

Rows are padded per scale to whole 128-row chunks; chunks are grouped
G at a time and stored [group][partition][G*85] so each group loads
with ONE DMA of 128 contiguous ~16KB packets (the original kernel was
DMA-descriptor-bound: 51k packets averaging 573B).

Device pipeline per group (no PE, no PSUM, one 128-lane row per
(cell,anchor)):
  - DVE : m  = reduce_max over the 80 class cols     [p, g, 80] f32
  - DVE : eq = (cls >= m) -> bf16 {0,1}  (exact: m is a member value)
  - DVE : eq *= iota (bf16 79-c); m2 = reduce_max(eq) = 79 - argmax
          (first-index ties win automatically: larger 79-c)
  - ACT : conf = Sigmoid(col0), ewh = Exp(cols 3:5)   (per-chunk
          grid offsets gx*t/416 and anchors/416 are shipped per
          (partition, chunk) so no per-scale special casing)
  - DVE : cx/cy fused stt, w/h = ewh*anchors, cls = 79 - m2,
          mask = (conf_logit > 0) * row   (one stt, 3D)
  - results accumulate in one SBUF-resident [128, 666*6] tile,
    flushed with a single 128x16KB DMA at the end.
"""

import os
from contextlib import ExitStack

import numpy as np

import concourse.bass as bass
import concourse.tile as tile
from concourse import bacc, mybir
from concourse.bass_utils import run_bass_kernel_spmd

N_CORES = 8
B = 64
B_PER = B // N_CORES
CASE = 416.0
SCALES = [("52", 52, 8.0), ("26", 26, 16.0), ("13", 13, 32.0)]
CHUNK = 128
ROW = 85           # f32 per (cell, anchor) row
G = int(os.environ.get("KGRP", "48"))
F32 = mybir.dt.float32
BF16 = mybir.dt.bfloat16
AX = mybir.AxisListType
OP = mybir.AluOpType
AF = mybir.ActivationFunctionType


def _cells(h):
    return B_PER * h * h


def _rows(h):
    return 3 * _cells(h)


def _nch(h):
    return (_rows(h) + CHUNK - 1) // CHUNK


NCH = {tag: _nch(h) for tag, h, _ in SCALES}     # 507, 127, 32
TOT_CH = sum(NCH.values())                        # 666
CH_OFF = {}
_c = 0
for _tag, _h, _t in SCALES:
    CH_OFF[_tag] = _c
    _c += NCH[_tag]

# groups: (tag, scale_idx, j0, gc, x_off_elems)
GROUPS = []
_off = 0
for _si, (_tag, _h, _t) in enumerate(SCALES):
    for _j0 in range(0, NCH[_tag], G):
        _gc = min(G, NCH[_tag] - _j0)
        GROUPS.append((_tag, _si, _j0, _gc, _off))
        _off += CHUNK * _gc * ROW
_GXY_OFF = _off
_off += CHUNK * TOT_CH * 2
_ANCH_OFF = _off
_off += CHUNK * TOT_CH * 2
_IOTA_OFF = _off
_off += CHUNK * 40                                # 80 bf16 packed as 40 f32
TOTAL_IN = _off
OUT_COLS = TOT_CH * 6


def _consts():
    """gxy / anch per (partition, global chunk): [128, TOT_CH, 2]."""
    import ml_dtypes
    gxy = np.zeros((CHUNK, TOT_CH, 2), np.float32)
    anch = np.zeros((CHUNK, TOT_CH, 2), np.float32)  # filled at pack time
    for si, (tag, h, t) in enumerate(SCALES):
        hw = h * h
        nr = _rows(h)
        nch = NCH[tag]
        r = np.arange(nch * CHUNK)
        cell = r // 3
        simg = cell % hw
        gx = (simg % h).astype(np.float64) * t / CASE
        gy = (simg // h).astype(np.float64) * t / CASE
        gx[r >= nr] = 0.0
        gy[r >= nr] = 0.0
        j0 = CH_OFF[tag]
        gxy[:, j0:j0 + nch, 0] = gx.reshape(nch, CHUNK).T
        gxy[:, j0:j0 + nch, 1] = gy.reshape(nch, CHUNK).T
    iota = np.zeros((CHUNK, 80), ml_dtypes.bfloat16)
    iota[:, :] = (79.0 - np.arange(80))[None, :]
    return gxy, iota.view(np.float32)


_GXY, _IOTA = _consts()


def _anch_pj(anchors):
    """[128, TOT_CH, 2] f32: anchors[a(r), d] / 416 per (p, chunk)."""
    anch = np.zeros((CHUNK, TOT_CH, 2), np.float32)
    for si, (tag, h, t) in enumerate(SCALES):
        nr = _rows(h)
        nch = NCH[tag]
        a416 = np.asarray(anchors[tag], np.float64) / CASE  # [3, 2]
        r = np.arange(nch * CHUNK)
        av = a416[r % 3]                                    # [nch*128, 2]
        av[r >= nr] = 0.0
        j0 = CH_OFF[tag]
        anch[:, j0:j0 + nch, :] = av.reshape(nch, CHUNK, 2) \
            .transpose(1, 0, 2).astype(np.float32)
    return anch


def build():
    nc = bacc.Bacc("TRN2", target_bir_lowering=False, debug=False,
                   num_devices=N_CORES)
    xin = nc.dram_tensor("xin", [TOTAL_IN], F32, kind="ExternalInput").ap()
    oX = nc.dram_tensor("out", [CHUNK, OUT_COLS], F32,
                        kind="ExternalOutput").ap()

    with tile.TileContext(nc) as tc:
        with ExitStack() as ctx:
            p_c = ctx.enter_context(tc.tile_pool(name="consts", bufs=1))
            p_in = ctx.enter_context(tc.tile_pool(name="inp", bufs=3))
            p_eq = ctx.enter_context(tc.tile_pool(name="eq", bufs=3))
            p_s = ctx.enter_context(tc.tile_pool(name="small", bufs=3))

            def load_const(name, cols, off):
                t_ = p_c.tile([CHUNK, cols], F32, tag=name)
                nc.sync.dma_start(
                    t_[:], xin[off:off + CHUNK * cols]
                    .rearrange("(p f) -> p f", p=CHUNK))
                return t_

            gxy_t = load_const("gxy", TOT_CH * 2, _GXY_OFF)
            anch_t = load_const("anch", TOT_CH * 2, _ANCH_OFF)
            iota_t = load_const("iota", 40, _IOTA_OFF)
            gxy_v = gxy_t[:].rearrange("p (j q) -> p j q", q=2)
            anch_v = anch_t[:].rearrange("p (j q) -> p j q", q=2)
            iota80 = iota_t[:].bitcast(BF16)                # [128, 80]

            out_t = p_c.tile([CHUNK, OUT_COLS], F32, tag="out_t")
            o_all = out_t[:].rearrange("p (ch s) -> p ch s", s=6)

            for tag, si, j0, gc, xoff in GROUPS:
                _, h, t = SCALES[si]
                k = float(t / CASE)
                jg = CH_OFF[tag] + j0                       # global chunk idx

                strip = p_in.tile([CHUNK, G * ROW], F32, tag="strip")
                nc.sync.dma_start(
                    strip[0:CHUNK, 0:gc * ROW],
                    xin[xoff:xoff + CHUNK * gc * ROW]
                    .rearrange("(p f) -> p f", p=CHUNK))
                sv = strip[:].rearrange("p (g c) -> p g c", g=G)[:, 0:gc]
                cls_ap = sv[:, :, 5:85]                     # [p, gc, 80]

                m = p_s.tile([CHUNK, G], F32, tag="m")
                m_v = m[:, 0:gc]
                nc.vector.tensor_reduce(m_v, cls_ap, axis=AX.X, op=OP.max)

                eq = p_eq.tile([CHUNK, G * 80], BF16, tag="eq")
                eq_v = eq[:].rearrange("p (g r) -> p g r", g=G)[:, 0:gc]
                m_b = m_v.unsqueeze(2).broadcast_to([CHUNK, gc, 80])
                nc.vector.tensor_tensor(eq_v, cls_ap, m_b, op=OP.is_ge)

                iota_b = iota80.unsqueeze(1).broadcast_to([CHUNK, gc, 80])
                nc.vector.tensor_tensor(eq_v, eq_v, iota_b, op=OP.mult)

                # second reduce as a tensor_tensor max tree: TT has a 2x
                # bf16 uop while tensor_reduce is 1x-only on DVE.
                w = 80
                while w > 5:
                    hw_ = w // 2
                    nc.vector.tensor_tensor(
                        eq_v[:, :, 0:hw_], eq_v[:, :, 0:hw_],
                        eq_v[:, :, hw_:2 * hw_], op=OP.max)
                    if w % 2:
                        nc.vector.tensor_tensor(
                            eq_v[:, :, 0:1], eq_v[:, :, 0:1],
                            eq_v[:, :, w - 1:w], op=OP.max)
                    w = hw_
                m2 = p_s.tile([CHUNK, G], BF16, tag="m2")
                m2_v = m2[:, 0:gc]
                nc.vector.tensor_reduce(m2_v, eq_v[:, :, 0:w], axis=AX.X,
                                        op=OP.max)

                o_v = o_all[:, jg:jg + gc]                  # [p, gc, 6]

                nc.scalar.activation(o_v[:, :, 0:1].squeeze(2),
                                     sv[:, :, 0:1].squeeze(2), AF.Sigmoid)

                nc.vector.scalar_tensor_tensor(
                    o_v[:, :, 1:3], sv[:, :, 1:3], k,
                    gxy_v[:, jg:jg + gc, :], op0=OP.mult, op1=OP.add)

                ewh = p_s.tile([CHUNK, G * 2], F32, tag="ewh")
                ewh_v = ewh[:].rearrange("p (g q) -> p g q", q=2)[:, 0:gc]
                nc.scalar.activation(ewh_v, sv[:, :, 3:5], AF.Exp)
                nc.vector.tensor_tensor(o_v[:, :, 3:5], ewh_v,
                                        anch_v[:, jg:jg + gc, :], op=OP.mult)

                nc.vector.tensor_scalar(o_v[:, :, 5:6].squeeze(2), m2_v,
                                        -1.0, 79.0, op0=OP.mult, op1=OP.add)

                conf_b = sv[:, :, 0:1].broadcast_to([CHUNK, gc, 6])
                nc.vector.scalar_tensor_tensor(
                    o_v, conf_b, 0.0, o_v, op0=OP.is_gt, op1=OP.mult)

                if j0 + gc == NCH[tag]:
                    # flush this scale's finished output columns so the
                    # store overlaps later scales' compute
                    c0 = CH_OFF[tag] * 6
                    c1 = (CH_OFF[tag] + NCH[tag]) * 6
                    nc.sync.dma_start(oX[:, c0:c1], out_t[:, c0:c1])
    nc.compile()
    return nc


_NC = None


def _get_nc():
    global _NC
    if _NC is None:
        _NC = build()
    return _NC


def _make_anch(anchors):
    return _anch_pj(anchors).reshape(CHUNK, TOT_CH * 2)


def _pack_core(xs, anch):
    """xs: {tag: [B_PER, 255, h, h] f32}; anch: [128, TOT_CH*2] f32."""
    parts = []
    for si, (tag, h, t) in enumerate(SCALES):
        hw = h * h
        nr = _rows(h)
        nch = NCH[tag]
        x = np.asarray(xs[tag]).reshape(B_PER, 255, hw)
        rows = np.zeros((nch * CHUNK, ROW), np.float32)
        rows[:nr] = x.transpose(0, 2, 1).reshape(nr, ROW)
        a = rows.reshape(nch, CHUNK, ROW)
        for j0 in range(0, nch, G):
            gc = min(G, nch - j0)
            parts.append(np.ascontiguousarray(
                a[j0:j0 + gc].transpose(1, 0, 2)).ravel())
    parts.append(_GXY.ravel())
    parts.append(np.asarray(anch, np.float32).ravel())
    parts.append(_IOTA.ravel())
    out = np.concatenate(parts)
    assert out.size == TOTAL_IN and out.dtype == np.float32
    return out


def _unpack(res):
    """res: list of per-core {"out": [128, OUT_COLS]} -> [681408, 6]."""
    parts = []
    for tag, h, _ in SCALES[::-1]:               # output order: 13, 26, 52
        nr = _rows(h)
        nch = NCH[tag]
        c0 = CH_OFF[tag] * 6
        for i in range(N_CORES):
            o = res[i]["out"][:, c0:c0 + nch * 6]
            parts.append(o.reshape(CHUNK, nch, 6).transpose(1, 0, 2)
                         .reshape(nch * CHUNK, 6)[:nr])
    return np.concatenate(parts, axis=0)


def kernel(out13, out26, out52, anchors13, anchors26, anchors52):
    nc = _get_nc()
    xs_all = {"13": np.asarray(out13), "26": np.asarray(out26),
              "52": np.asarray(out52)}
    anchors = {"13": np.asarray(anchors13), "26": np.asarray(anchors26),
               "52": np.asarray(anchors52)}
    anch = _make_anch(anchors)

    in_maps = []
    for i in range(N_CORES):
        xs = {tag: xs_all[tag][i * B_PER:(i + 1) * B_PER]
              for tag, _, _ in SCALES}
        in_maps.append({"xin": _pack_core(xs, anch)})

    res = run_bass_kernel_spmd(nc, in_maps, list(range(N_CORES))).results
    return _unpack(res)


# revision 9
# speedup vs baseline: 396.6604x; 1.0109x over previous
"""YOLO-style detection decode (nms_detection) on 8 trn2 NeuronCores.

Data-parallel over batch (64 -> 8 images/core). The host repacks each
core's inputs into (cell, anchor)-major rows of 85 f32 — which is
exactly the raw per-anchor channel block [conf, x, y, w, h, cls x80] —
so the device needs NO transposes and every operand is a dense <=3D
access pattern:

  DRAM row r = cell*3 + a  (85 f32, 340B)

Rows are padded per scale to whole 128-row chunks; chunks are grouped
G at a time and stored [group][partition][G*85] so each group loads
with ONE DMA of 128 contiguous ~16KB packets (the original kernel was
DMA-descriptor-bound: 51k packets averaging 573B).

Device pipeline per group (no PE, no PSUM, one 128-lane row per
(cell,anchor)):
  - DVE : m  = reduce_max over the 80 class cols     [p, g, 80] f32
  - DVE : eq = (cls >= m) -> bf16 {0,1}  (exact: m is a member value)
  - DVE : eq *= iota (bf16 79-c); m2 = reduce_max(eq) = 79 - argmax
          (first-index ties win automatically: larger 79-c)
  - ACT : conf = Sigmoid(col0), ewh = Exp(cols 3:5)   (per-chunk
          grid offsets gx*t/416 and anchors/416 are shipped per
          (partition, chunk) so no per-scale special casing)
  - DVE : cx/cy fused stt, w/h = ewh*anchors, cls = 79 - m2,
          mask = (conf_logit > 0) * row   (one stt, 3D)
  - results accumulate in one SBUF-resident [128, 666*6] tile,
    flushed with a single 128x16KB DMA at the end.
"""

import os
from contextlib import ExitStack

import numpy as np

import concourse.bass as bass
import concourse.tile as tile
from concourse import bacc, mybir
from concourse.bass_utils import run_bass_kernel_spmd

N_CORES = 8
B = 64
B_PER = B // N_CORES
CASE = 416.0
SCALES = [("52", 52, 8.0), ("26", 26, 16.0), ("13", 13, 32.0)]
CHUNK = 128
ROW = 85           # f32 per (cell, anchor) row
G = int(os.environ.get("KGRP", "64"))
F32 = mybir.dt.float32
BF16 = mybir.dt.bfloat16
AX = mybir.AxisListType
OP = mybir.AluOpType
AF = mybir.ActivationFunctionType


def _cells(h):
    return B_PER * h * h


def _rows(h):
    return 3 * _cells(h)


def _nch(h):
    return (_rows(h) + CHUNK - 1) // CHUNK


NCH = {tag: _nch(h) for tag, h, _ in SCALES}     # 507, 127, 32
TOT_CH = sum(NCH.values())                        # 666
CH_OFF = {}
_c = 0
for _tag, _h, _t in SCALES:
    CH_OFF[_tag] = _c
    _c += NCH[_tag]

# groups: (tag, scale_idx, j0, gc, x_off_elems)
GROUPS = []
_off = 0
for _si, (_tag, _h, _t) in enumerate(SCALES):
    for _j0 in range(0, NCH[_tag], G):
        _gc = min(G, NCH[_tag] - _j0)
        GROUPS.append((_tag, _si, _j0, _gc, _off))
        _off += CHUNK * _gc * ROW
_GXY_OFF = _off
_off += CHUNK * TOT_CH * 2
_ANCH_OFF = _off
_off += CHUNK * TOT_CH * 2
_IOTA_OFF = _off
_off += CHUNK * 40                                # 80 bf16 packed as 40 f32
TOTAL_IN = _off
OUT_COLS = TOT_CH * 6


def _consts():
    """gxy / anch per (partition, global chunk): [128, TOT_CH, 2]."""
    import ml_dtypes
    gxy = np.zeros((CHUNK, TOT_CH, 2), np.float32)
    anch = np.zeros((CHUNK, TOT_CH, 2), np.float32)  # filled at pack time
    for si, (tag, h, t) in enumerate(SCALES):
        hw = h * h
        nr = _rows(h)
        nch = NCH[tag]
        r = np.arange(nch * CHUNK)
        cell = r // 3
        simg = cell % hw
        gx = (simg % h).astype(np.float64) * t / CASE
        gy = (simg // h).astype(np.float64) * t / CASE
        gx[r >= nr] = 0.0
        gy[r >= nr] = 0.0
        j0 = CH_OFF[tag]
        gxy[:, j0:j0 + nch, 0] = gx.reshape(nch, CHUNK).T
        gxy[:, j0:j0 + nch, 1] = gy.reshape(nch, CHUNK).T
    iota = np.zeros((CHUNK, 80), ml_dtypes.bfloat16)
    iota[:, :] = (79.0 - np.arange(80))[None, :]
    return gxy, iota.view(np.float32)


_GXY, _IOTA = _consts()


def _anch_pj(anchors):
    """[128, TOT_CH, 2] f32: anchors[a(r), d] / 416 per (p, chunk)."""
    anch = np.zeros((CHUNK, TOT_CH, 2), np.float32)
    for si, (tag, h, t) in enumerate(SCALES):
        nr = _rows(h)
        nch = NCH[tag]
        a416 = np.asarray(anchors[tag], np.float64) / CASE  # [3, 2]
        r = np.arange(nch * CHUNK)
        av = a416[r % 3]                                    # [nch*128, 2]
        av[r >= nr] = 0.0
        j0 = CH_OFF[tag]
        anch[:, j0:j0 + nch, :] = av.reshape(nch, CHUNK, 2) \
            .transpose(1, 0, 2).astype(np.float32)
    return anch


def build():
    nc = bacc.Bacc("TRN2", target_bir_lowering=False, debug=False,
                   num_devices=N_CORES)
    xin = nc.dram_tensor("xin", [TOTAL_IN], F32, kind="ExternalInput").ap()
    oX = nc.dram_tensor("out", [CHUNK, OUT_COLS], F32,
                        kind="ExternalOutput").ap()

    with tile.TileContext(nc) as tc:
        with ExitStack() as ctx:
            p_c = ctx.enter_context(tc.tile_pool(name="consts", bufs=1))
            p_in = ctx.enter_context(tc.tile_pool(name="inp", bufs=3))
            p_eq = ctx.enter_context(tc.tile_pool(name="eq", bufs=3))
            p_s = ctx.enter_context(tc.tile_pool(name="small", bufs=3))

            def load_const(name, cols, off):
                t_ = p_c.tile([CHUNK, cols], F32, tag=name)
                nc.sync.dma_start(
                    t_[:], xin[off:off + CHUNK * cols]
                    .rearrange("(p f) -> p f", p=CHUNK))
                return t_

            gxy_t = load_const("gxy", TOT_CH * 2, _GXY_OFF)
            anch_t = load_const("anch", TOT_CH * 2, _ANCH_OFF)
            iota_t = load_const("iota", 40, _IOTA_OFF)
            gxy_v = gxy_t[:].rearrange("p (j q) -> p j q", q=2)
            anch_v = anch_t[:].rearrange("p (j q) -> p j q", q=2)
            iota80 = iota_t[:].bitcast(BF16)                # [128, 80]

            out_t = p_c.tile([CHUNK, OUT_COLS], F32, tag="out_t")
            o_all = out_t[:].rearrange("p (ch s) -> p ch s", s=6)

            for tag, si, j0, gc, xoff in GROUPS:
                _, h, t = SCALES[si]
                k = float(t / CASE)
                jg = CH_OFF[tag] + j0                       # global chunk idx

                strip = p_in.tile([CHUNK, G * ROW], F32, tag="strip")
                nc.sync.dma_start(
                    strip[0:CHUNK, 0:gc * ROW],
                    xin[xoff:xoff + CHUNK * gc * ROW]
                    .rearrange("(p f) -> p f", p=CHUNK))
                sv = strip[:].rearrange("p (g c) -> p g c", g=G)[:, 0:gc]
                cls_ap = sv[:, :, 5:85]                     # [p, gc, 80]

                m = p_s.tile([CHUNK, G], F32, tag="m")
                m_v = m[:, 0:gc]
                nc.vector.tensor_reduce(m_v, cls_ap, axis=AX.X, op=OP.max)

                eq = p_eq.tile([CHUNK, G * 80], BF16, tag="eq")
                eq_v = eq[:].rearrange("p (g r) -> p g r", g=G)[:, 0:gc]
                m_b = m_v.unsqueeze(2).broadcast_to([CHUNK, gc, 80])
                nc.vector.tensor_tensor(eq_v, cls_ap, m_b, op=OP.is_ge)

                iota_b = iota80.unsqueeze(1).broadcast_to([CHUNK, gc, 80])
                nc.vector.tensor_tensor(eq_v, eq_v, iota_b, op=OP.mult)

                # second reduce as a tensor_tensor max tree: TT has a 2x
                # bf16 uop while tensor_reduce is 1x-only on DVE.
                w = 80
                while w > 5:
                    hw_ = w // 2
                    nc.vector.tensor_tensor(
                        eq_v[:, :, 0:hw_], eq_v[:, :, 0:hw_],
                        eq_v[:, :, hw_:2 * hw_], op=OP.max)
                    if w % 2:
                        nc.vector.tensor_tensor(
                            eq_v[:, :, 0:1], eq_v[:, :, 0:1],
                            eq_v[:, :, w - 1:w], op=OP.max)
                    w = hw_
                m2 = p_s.tile([CHUNK, G], BF16, tag="m2")
                m2_v = m2[:, 0:gc]
                nc.vector.tensor_reduce(m2_v, eq_v[:, :, 0:w], axis=AX.X,
                                        op=OP.max)

                o_v = o_all[:, jg:jg + gc]                  # [p, gc, 6]

                nc.scalar.activation(o_v[:, :, 0:1].squeeze(2),
                                     sv[:, :, 0:1].squeeze(2), AF.Sigmoid)

                nc.vector.scalar_tensor_tensor(
                    o_v[:, :, 1:3], sv[:, :, 1:3], k,
                    gxy_v[:, jg:jg + gc, :], op0=OP.mult, op1=OP.add)

                ewh = p_s.tile([CHUNK, G * 2], F32, tag="ewh")
                ewh_v = ewh[:].rearrange("p (g q) -> p g q", q=2)[:, 0:gc]
                nc.scalar.activation(ewh_v, sv[:, :, 3:5], AF.Exp)
                nc.vector.tensor_tensor(o_v[:, :, 3:5], ewh_v,
                                        anch_v[:, jg:jg + gc, :], op=OP.mult)

                nc.vector.tensor_scalar(o_v[:, :, 5:6].squeeze(2), m2_v,
                                        -1.0, 79.0, op0=OP.mult, op1=OP.add)

                conf_b = sv[:, :, 0:1].broadcast_to([CHUNK, gc, 6])
                nc.vector.scalar_tensor_tensor(
                    o_v, conf_b, 0.0, o_v, op0=OP.is_gt, op1=OP.mult)

                if j0 + gc == NCH[tag]:
                    # flush this scale's finished output columns so the
                    # store overlaps later scales' compute
                    c0 = CH_OFF[tag] * 6
                    c1 = (CH_OFF[tag] + NCH[tag]) * 6
                    nc.sync.dma_start(oX[:, c0:c1], out_t[:, c0:c1])
    nc.compile()
    return nc


_NC = None


def _get_nc():
    global _NC
    if _NC is None:
        _NC = build()
    return _NC


def _make_anch(anchors):
    return _anch_pj(anchors).reshape(CHUNK, TOT_CH * 2)


def _pack_core(xs, anch):
    """xs: {tag: [B_PER, 255, h, h] f32}; anch: [128, TOT_CH*2] f32."""
    parts = []
    for si, (tag, h, t) in enumerate(SCALES):
        hw = h * h
        nr = _rows(h)
        nch = NCH[tag]
        x = np.asarray(xs[tag]).reshape(B_PER, 255, hw)
        rows = np.zeros((nch * CHUNK, ROW), np.float32)
        rows[:nr] = x.transpose(0, 2, 1).reshape(nr, ROW)
        a = rows.reshape(nch, CHUNK, ROW)
        for j0 in range(0, nch, G):
            gc = min(G, nch - j0)
            parts.append(np.ascontiguousarray(
                a[j0:j0 + gc].transpose(1, 0, 2)).ravel())
    parts.append(_GXY.ravel())
    parts.append(np.asarray(anch, np.float32).ravel())
    parts.append(_IOTA.ravel())
    out = np.concatenate(parts)
    assert out.size == TOTAL_IN and out.dtype == np.float32
    return out


def _unpack(res):
    """res: list of per-core {"out": [128, OUT_COLS]} -> [681408, 6]."""
    parts = []
    for tag, h, _ in SCALES[::-1]:               # output order: 13, 26, 52
        nr = _rows(h)
        nch = NCH[tag]
        c0 = CH_OFF[tag] * 6
        for i in range(N_CORES):
            o = res[i]["out"][:, c0:c0 + nch * 6]
            parts.append(o.reshape(CHUNK, nch, 6).transpose(1, 0, 2)
                         .reshape(nch * CHUNK, 6)[:nr])
    return np.concatenate(parts, axis=0)


def kernel(out13, out26, out52, anchors13, anchors26, anchors52):
    nc = _get_nc()
    xs_all = {"13": np.asarray(out13), "26": np.asarray(out26),
              "52": np.asarray(out52)}
    anchors = {"13": np.asarray(anchors13), "26": np.asarray(anchors26),
               "52": np.asarray(anchors52)}
    anch = _make_anch(anchors)

    in_maps = []
    for i in range(N_CORES):
        xs = {tag: xs_all[tag][i * B_PER:(i + 1) * B_PER]
              for tag, _, _ in SCALES}
        in_maps.append({"xin": _pack_core(xs, anch)})

    res = run_bass_kernel_spmd(nc, in_maps, list(range(N_CORES))).results
    return _unpack(res)
